# revision 26
# baseline (speedup 1.0000x reference)
"""Bezier Gaussian-splat raster kernel for 8 Trainium2 NeuronCores.

Reference computation (RES=1024, STEPS=256, SIGMA=0.01):
    curve = bezier(control_points)            # (2, 256)
    Ex[a,s] = exp(-(g[a]-x[s])^2 / (2 sigma^2))   # (1024, 256)
    Ey[b,s] = exp(-(g[b]-y[s])^2 / (2 sigma^2))
    OUT     = (Ey @ Ex^T) / 256               # (1024, 1024)  == raster.T

Sharding: 4 row-blocks x 2 col-blocks = 8 cores. Core i handles output rows
[256*(i//2), +256) and cols [512*(i%2), +512); the block index comes from the
partition_id input, so all cores run identical code on identical inputs.

The 6 control-point floats are fetched with a ~1µs sequencer register load
(bypassing the ~4µs DMA completion latency), written to SBUF, and broadcast
across partitions with a K=1 matmul. Everything else (grid coordinates,
Bezier basis) is generated on device from iota. Per-core offsets are applied
to the *curve* (x' = x - offs), never to the grids, so the whole grid /
-c*g^2 pipeline is independent of the input fetch. The 1/STEPS scale and
-c*g^2 completion terms are folded into the exponent terms, so the fp16
matmul accumulation is the final output.

Engine split: GpSimd iota+grid-scale | DVE basis/curve/coeffs/exponent-args |
ACT squares + exp (one table load) + evac | PE HAM-warmup dummies, broadcast,
2x2 fp16 matmuls | SP input register fetch + store ring.
"""

import math

import numpy as np

import concourse.bacc as bacc
import concourse.bass as bass
import concourse.mybir as mybir
import concourse.tile as tile
from concourse.bass_utils import run_bass_kernel_spmd

RES = 1024
STEPS = 256
SIGMA = 0.01
INV2S2 = 1.0 / (2.0 * SIGMA * SIGMA)  # 5000.0
SQC = math.sqrt(INV2S2)
LN_S = math.log(STEPS)
FGEN = 0x4B000000  # float32 bits of 2^23; +t encodes float(2^23 + t)

R_BLK = 4
C_BLK = 2
MROWS = RES // R_BLK  # 256
NCOLS = RES // C_BLK  # 512
N_CORES = 8

F32 = mybir.dt.float32
U32 = mybir.dt.uint32
F16 = mybir.dt.float16
I16 = mybir.dt.int16

G_DTYPE = F16

_CACHE: dict = {}


def _build_nc() -> bass.Bass:
    # Skip the ~3µs all-engine EVSEM barrier Bass.__init__ emits after its
    # const-AP memsets; our first const-AP use is µs later.
    _orig_barrier = bass.Bass.all_engine_barrier
    bass.Bass.all_engine_barrier = lambda self, **kw: None
    try:
        nc = bacc.Bacc(
            "TRN2",
            target_bir_lowering=False,
            debug=False,
            enable_asserts=False,
            enable_partition_id=True,
        )
    finally:
        bass.Bass.all_engine_barrier = _orig_barrier

    # control_points as raw fp32 bits (TENSOR_LOAD needs an int source)
    cp6 = nc.dram_tensor("cp6", [1, 8], U32, kind="ExternalInput").ap()
    out = nc.dram_tensor("out", [MROWS, NCOLS], F32, kind="ExternalOutput").ap()

    MULT = mybir.AluOpType.mult
    ADD = mybir.AluOpType.add
    SUB = mybir.AluOpType.subtract
    EXP = mybir.ActivationFunctionType.Exp
    SQUARE = mybir.ActivationFunctionType.Square

    with tile.TileContext(nc) as tc:
        with (
            tc.tile_pool(name="const", bufs=1) as cpool,
            tc.tile_pool(name="work", bufs=1) as wpool,
            tc.tile_pool(name="ps", bufs=1, space="PSUM") as ppool,
        ):
            # === SP: fetch control points into registers, store to SBUF ===
            cprow = cpool.tile([1, 8], mybir.dt.int32)
            regs = [nc.sync.alloc_register(f"cpr{i}") for i in range(6)]
            nc.sync.reg_load(regs, cp6[0:1, 0:6])
            for j in range(6):
                nc.sync.store(cprow[0:1, j : j + 1], regs[j])

            # === ACT: partition id -> float-gen offset bits ================
            pidr = nc.scalar.alloc_register("pidr")
            nc.scalar.reg_load(pidr, nc.partition_id_tensor.ap()[0:1, 0:1])
            t0 = nc.scalar.alloc_register("t0")
            nc.scalar.reg_alu(t0, pidr, 1, mybir.AluOpType.bitwise_and)
            ox = nc.scalar.alloc_register("ox")
            nc.scalar.reg_alu(ox, t0, FGEN, ADD)
            t1 = nc.scalar.alloc_register("t1")
            nc.scalar.reg_alu(t1, pidr, 1, mybir.AluOpType.logical_shift_right)
            oy = nc.scalar.alloc_register("oy")
            nc.scalar.reg_alu(oy, t1, FGEN, ADD)
            nc.scalar.store(cprow[0:1, 6:7], ox)
            nc.scalar.store(cprow[0:1, 7:8], oy)

            # early exp-table load trigger (exp set also has square/identity)
            scratch = cpool.tile([128, 2], F32)
            nc.gpsimd.memset(scratch[:], 0.0)
            nc.scalar.activation(scratch[:, 1:2], scratch[:, 0:1], EXP)

            # === PE: warmup + broadcast ===================================
            ones = cpool.tile([1, 128], F32)
            nc.gpsimd.memset(ones[:], 1.0)
            wdum = cpool.tile([128, 128], G_DTYPE)
            nc.gpsimd.memset(wdum[:], 0.25)
            pdum = ppool.tile([128, 128], F32)
            for _ in range(8):
                nc.tensor.matmul(pdum[:], wdum[:], wdum[:], start=True, stop=True)
            # cpb[p, 0:6] = control points, [p, 6:8] = offset float-gen bits
            cpb = ppool.tile([128, 8], F32)
            nc.tensor.matmul(
                cpb[:], ones[:], cprow.bitcast(F32)[:], start=True, stop=True
            )
            for _ in range(44):
                nc.tensor.matmul(pdum[:], wdum[:], wdum[:], start=True, stop=True)

            # === GpSimd: iota grids + scale ================================
            sPk = cpool.tile([128, 2], I16)
            nc.gpsimd.iota(sPk[:], [[128, 2]], base=0, channel_multiplier=1)
            gxi = cpool.tile([128, NCOLS], I16)
            nc.gpsimd.iota(gxi[:], [[1, NCOLS]], base=0, channel_multiplier=0)
            gyi = cpool.tile([128, MROWS], I16)
            nc.gpsimd.iota(gyi[:], [[1, MROWS]], base=0, channel_multiplier=0)
            gx_sb = wpool.tile([128, NCOLS], F32, tag="gx")
            nc.gpsimd.tensor_scalar(gx_sb[:], gxi[:], 1.0 / RES, None, MULT)
            gy_sb = wpool.tile([128, MROWS], F32, tag="gy")
            nc.gpsimd.tensor_scalar(gy_sb[:], gyi[:], 1.0 / RES, None, MULT)

            # === ACT: +c*g^2 via Square(sqrt(c)*g) ========================
            cg2x = wpool.tile([128, NCOLS], F32, tag="cg2x")
            nc.scalar.activation(cg2x[:], gx_sb[:], SQUARE, scale=SQC)
            cg2y = wpool.tile([128, MROWS], F32, tag="cg2y")
            nc.scalar.activation(cg2y[:], gy_sb[:], SQUARE, scale=SQC)

            # === DVE: Bezier basis (s = 128k + p on partitions) ===========
            # B3[p, 2j+k] = basis_j(s); u = s/255 (linspace), v = s/256
            u = wpool.tile([128, 2], F32)
            nc.vector.tensor_scalar(u[:], sPk[:], 1.0 / 255.0, None, MULT)
            v = wpool.tile([128, 2], F32)
            nc.vector.tensor_scalar(v[:], sPk[:], 1.0 / 256.0, None, MULT)
            su = wpool.tile([128, 2], F32)
            nc.vector.tensor_scalar(su[:], u[:], -1.0, 1.0, MULT, ADD)
            sv = wpool.tile([128, 2], F32)
            nc.vector.tensor_scalar(sv[:], v[:], -1.0, 1.0, MULT, ADD)
            B3 = wpool.tile([128, 6], F32)
            nc.vector.tensor_tensor(B3[:, 0:2], su[:], sv[:], MULT)  # c0
            nc.vector.tensor_tensor(B3[:, 4:6], u[:], v[:], MULT)  # c2
            c02 = wpool.tile([128, 2], F32)
            nc.vector.tensor_tensor(c02[:], B3[:, 0:2], B3[:, 4:6], ADD)
            nc.vector.tensor_scalar(B3[:, 2:4], c02[:], -1.0, 1.0, MULT, ADD)  # c1

            # === DVE: offsets, curve points, coefficients =================
            offs = wpool.tile([128, 2], F32)
            nc.vector.tensor_scalar(
                offs[:, 0:1], cpb[:, 6:7], -8388608.0, 1.0 / C_BLK, ADD, MULT
            )
            nc.vector.tensor_scalar(
                offs[:, 1:2], cpb[:, 7:8], -8388608.0, 1.0 / R_BLK, ADD, MULT
            )
            # prods[p, k*6 + j*2 + d] = basis_j(s_k) * cp[j, d]
            b3a = B3[:, 0:6]
            in0 = bass.AP(
                b3a.tensor, b3a.offset, [list(b3a.ap[0]), [1, 2], [2, 3], [0, 2]]
            )
            cba = cpb[:, 0:8]
            in1 = bass.AP(
                cba.tensor, cba.offset, [list(cba.ap[0]), [0, 2], [2, 3], [1, 2]]
            )
            prods = wpool.tile([128, 12], F32)
            nc.vector.tensor_tensor(prods[:], in0, in1, MULT)
            pv = prods.rearrange("p (k j d) -> p k j d", k=2, j=3)
            t12 = wpool.tile([128, 4], F32)
            nc.vector.tensor_tensor(t12[:], pv[:, :, 0:1, :], pv[:, :, 1:2, :], ADD)
            # xy4[p, 2k+d] = curve - block offset (grids stay block-local)
            xy4 = wpool.tile([128, 4], F32)
            oa = offs[:, 0:2]
            offs_rep = bass.AP(oa.tensor, oa.offset, [list(oa.ap[0]), [0, 2], [1, 2]])
            t3 = wpool.tile([128, 4], F32)
            nc.vector.tensor_tensor(t3[:], t12[:], pv[:, :, 2:3, :], ADD)
            nc.vector.tensor_tensor(xy4[:], t3[:], offs_rep, SUB)

            # bc[:, 0:4] = B = 2c*xy' ; bc[:, 4:8] = C2 = -c*xy'^2 (-lnS on y)
            bc = wpool.tile([128, 8], F32)
            nc.vector.tensor_scalar(bc[:, 0:4], xy4[:], 2.0 * INV2S2, None, MULT)
            nc.vector.scalar_tensor_tensor(
                bc[:, 4:8], xy4[:], -INV2S2, xy4[:], MULT, MULT
            )
            nc.vector.tensor_scalar(bc[:, 5:6], bc[:, 5:6], LN_S, None, SUB)
            nc.vector.tensor_scalar(bc[:, 7:8], bc[:, 7:8], LN_S, None, SUB)

            # === args + exp ===============================================
            gxe = []
            gye = []
            for k in range(2):
                argx = wpool.tile([128, NCOLS], F32, tag=f"argx{k}")
                nc.vector.scalar_tensor_tensor(
                    argx[:], gx_sb[:], bc[:, 2 * k : 2 * k + 1], cg2x[:], MULT, SUB
                )
                ex = wpool.tile([128, NCOLS], G_DTYPE, tag=f"gxe{k}")
                nc.scalar.activation(
                    ex[:], argx[:], EXP, bias=bc[:, 4 + 2 * k : 5 + 2 * k]
                )
                gxe.append(ex)

                argy = wpool.tile([128, MROWS], F32, tag=f"argy{k}")
                nc.vector.scalar_tensor_tensor(
                    argy[:], gy_sb[:], bc[:, 2 * k + 1 : 2 * k + 2], cg2y[:],
                    MULT, SUB
                )
                ey = wpool.tile([128, MROWS], G_DTYPE, tag=f"gye{k}")
                nc.scalar.activation(
                    ey[:], argy[:], EXP, bias=bc[:, 5 + 2 * k : 6 + 2 * k]
                )
                gye.append(ey)

            # === matmul: OUT[m, n] = sum_s Ey[s, m] * Ex[s, n] =============
            pouts = [
                ppool.tile([128, NCOLS], F32, tag=f"pout{m}", name=f"pout{m}")
                for m in range(2)
            ]
            for k in range(2):
                for m in (1, 0):
                    nc.tensor.matmul(
                        pouts[m][:],
                        gye[k][:, 128 * m : 128 * (m + 1)],
                        gxe[k][:],
                        start=(k == 0),
                        stop=(k == 1),
                        skip_group_check=True,
                    )

            # === evacuate + store =========================================
            out1 = wpool.tile([128, NCOLS], F32, tag="out1")
            nc.vector.tensor_copy(out1[:], pouts[1][:])
            nc.sync.dma_start(out[128:256, :], out1[:])
            out0 = wpool.tile([128, NCOLS], F32, tag="out0")
            nc.scalar.copy(out0[:], pouts[0][:])
            nc.scalar.dma_start(out[0:128, :], out0[:])

    nc.compile()
    return nc


def _get_cached():
    if "nc" not in _CACHE:
        _CACHE["nc"] = _build_nc()
    return _CACHE["nc"]


def kernel(control_points: np.ndarray, _trace: bool = False):
    nc = _get_cached()
    cp = np.asarray(control_points, dtype=np.float32)
    assert cp.shape == (3, 2)

    cp6 = np.zeros((1, 8), dtype=np.uint32)
    cp6[0, 0:6] = np.ascontiguousarray(cp.reshape(-1)).view(np.uint32)
    in_maps = [{"cp6": cp6} for _ in range(N_CORES)]

    res = run_bass_kernel_spmd(
        nc, in_maps, core_ids=list(range(N_CORES)), trace=_trace
    )
    _CACHE["last_results"] = res

    full = np.empty((RES, RES), dtype=np.float32)
    for i in range(N_CORES):
        r, c = i // C_BLK, i % C_BLK
        full[r * MROWS : (r + 1) * MROWS, c * NCOLS : (c + 1) * NCOLS] = res.results[
            i
        ]["out"]
    return full


# revision 27
# speedup vs baseline: 1.5590x; 1.5590x over previous
"""Bezier Gaussian-splat raster kernel for 8 Trainium2 NeuronCores.

Reference computation (RES=1024, STEPS=256, SIGMA=0.01):
    curve = bezier(control_points)            # (2, 256)
    Ex[a,s] = exp(-(g[a]-x[s])^2 / (2 sigma^2))   # (1024, 256)
    Ey[b,s] = exp(-(g[b]-y[s])^2 / (2 sigma^2))
    OUT     = (Ey @ Ex^T) / 256               # (1024, 1024)  == raster.T

Sharding: 4 row-blocks x 2 col-blocks = 8 cores. Core i handles output rows
[256*(i//2), +256) and cols [512*(i%2), +512).

Design notes (per core):
  - One tiny input DMA: control points broadcast to 128 partitions plus the
    block offsets, [128, 16] f32. The grid pipeline never touches it: grids
    are iota-generated block-LOCAL indices; the per-core offset is applied to
    the curve instead (x' = x - offs), which is mathematically identical.
  - Bezier basis is computed on device from a [128, 2] iota; curve points are
    elementwise basis*control-point products summed on DVE.
  - Exponent args stay fp32: arg = (2c x'/RES)*j - Square(sqrt(c)/RES * j)
    (+ per-point bias -c x'^2 inside the ACT exp). exp outputs are fp16.
  - The 1/STEPS scale rides the y-side exp biases (-ln S).
  - 256-contraction fp16 matmuls (2 s-chunks x 2 m-chunks, N=512) write the
    final output into PSUM; ACT and DVE evacuate one m-chunk each and the two
    stores go out on the two HWDGE rings in parallel.
"""

import math

import numpy as np

import concourse.bacc as bacc
import concourse.bass as bass
import concourse.mybir as mybir
import concourse.tile as tile
from concourse.bass_utils import run_bass_kernel_spmd

RES = 1024
STEPS = 256
SIGMA = 0.01
INV2S2 = 1.0 / (2.0 * SIGMA * SIGMA)  # 5000.0
SQC = math.sqrt(INV2S2)
LN_S = math.log(STEPS)

R_BLK = 4
C_BLK = 2
MROWS = RES // R_BLK  # 256
NCOLS = RES // C_BLK  # 512
N_CORES = 8

F32 = mybir.dt.float32
F16 = mybir.dt.float16
I16 = mybir.dt.int16

G_DTYPE = F16

_CACHE: dict = {}


def _build_nc() -> bass.Bass:
    # Skip the ~3µs all-engine EVSEM barrier Bass.__init__ emits after its
    # const-AP memsets; our first const-AP use is µs later.
    _orig_barrier = bass.Bass.all_engine_barrier
    bass.Bass.all_engine_barrier = lambda self, **kw: None
    try:
        nc = bacc.Bacc(
            "TRN2",
            target_bir_lowering=False,
            debug=False,
            enable_asserts=False,
            enable_partition_id=False,
        )
    finally:
        bass.Bass.all_engine_barrier = _orig_barrier

    # cols 0:12 control points k-duplicated (cp[j,d] at k*6+j*2+d),
    # cols 12:14 block offsets (x, y), rest pad.
    cpk = nc.dram_tensor("cpk", [128, 16], F32, kind="ExternalInput").ap()
    out = nc.dram_tensor("out", [MROWS, NCOLS], F32, kind="ExternalOutput").ap()

    MULT = mybir.AluOpType.mult
    ADD = mybir.AluOpType.add
    SUB = mybir.AluOpType.subtract
    EXP = mybir.ActivationFunctionType.Exp
    SQUARE = mybir.ActivationFunctionType.Square

    with tile.TileContext(nc) as tc:
        with (
            tc.tile_pool(name="const", bufs=1) as cpool,
            tc.tile_pool(name="work", bufs=1) as wpool,
            tc.tile_pool(name="ps", bufs=1, space="PSUM") as ppool,
        ):
            # --- the one input DMA, on the ACT HWDGE ring, issued first ----
            cpk_sb = cpool.tile([128, 16], F32)
            nc.scalar.dma_start(cpk_sb[:], cpk)

            # --- early ACT exp-table load trigger --------------------------
            scratch = cpool.tile([128, 2], F32)
            nc.gpsimd.memset(scratch[:], 0.0)
            nc.scalar.activation(scratch[:, 1:2], scratch[:, 0:1], EXP)

            # --- iota grids (int16 indices, block-local) -------------------
            sPk = cpool.tile([128, 2], I16)
            nc.gpsimd.iota(sPk[:], [[128, 2]], base=0, channel_multiplier=1)
            gxi = cpool.tile([128, NCOLS], I16)
            nc.gpsimd.iota(gxi[:], [[1, NCOLS]], base=0, channel_multiplier=0)
            gyi = cpool.tile([128, MROWS], I16)
            nc.gpsimd.iota(gyi[:], [[1, MROWS]], base=0, channel_multiplier=0)

            # --- +c*(j/RES)^2 via ACT Square -------------------------------
            cg2x = wpool.tile([128, NCOLS], F32, tag="cg2x")
            nc.scalar.activation(cg2x[:], gxi[:], SQUARE, scale=SQC / RES)
            cg2y = wpool.tile([128, MROWS], F32, tag="cg2y")
            nc.scalar.activation(cg2y[:], gyi[:], SQUARE, scale=SQC / RES)

            # --- Bezier basis on DVE (s = 128k + p) ------------------------
            # B3[p, 2j+k] = basis_j(s); u = s/255 (linspace), v = s/256
            u = wpool.tile([128, 2], F32)
            nc.vector.tensor_scalar(u[:], sPk[:], 1.0 / 255.0, None, MULT)
            v = wpool.tile([128, 2], F32)
            nc.vector.tensor_scalar(v[:], sPk[:], 1.0 / 256.0, None, MULT)
            su = wpool.tile([128, 2], F32)
            nc.vector.tensor_scalar(su[:], u[:], -1.0, 1.0, MULT, ADD)
            sv = wpool.tile([128, 2], F32)
            nc.vector.tensor_scalar(sv[:], v[:], -1.0, 1.0, MULT, ADD)
            B3 = wpool.tile([128, 6], F32)
            nc.vector.tensor_tensor(B3[:, 0:2], su[:], sv[:], MULT)  # c0
            nc.vector.tensor_tensor(B3[:, 4:6], u[:], v[:], MULT)  # c2
            c02 = wpool.tile([128, 2], F32)
            nc.vector.tensor_tensor(c02[:], B3[:, 0:2], B3[:, 4:6], ADD)
            nc.vector.tensor_scalar(B3[:, 2:4], c02[:], -1.0, 1.0, MULT, ADD)  # c1

            # --- curve points (shifted by block offsets) -------------------
            # prods[p, k*6+j*2+d] = basis_j(s_k) * cp[j, d]
            b3a = B3[:, 0:6]
            in0 = bass.AP(
                b3a.tensor, b3a.offset, [list(b3a.ap[0]), [1, 2], [2, 3], [0, 2]]
            )
            prods = wpool.tile([128, 12], F32)
            nc.vector.tensor_tensor(prods[:], in0, cpk_sb[:, 0:12], MULT)
            pv = prods.rearrange("p (k j d) -> p k j d", k=2, j=3)
            t12 = wpool.tile([128, 4], F32)
            nc.vector.tensor_tensor(t12[:], pv[:, :, 0:1, :], pv[:, :, 1:2, :], ADD)
            t3 = wpool.tile([128, 4], F32)
            nc.vector.tensor_tensor(t3[:], t12[:], pv[:, :, 2:3, :], ADD)
            # xy4[p, 2k+d] = curve - block offset  (grids stay block-local)
            oa = cpk_sb[:, 12:14]
            offs_rep = bass.AP(oa.tensor, oa.offset, [list(oa.ap[0]), [0, 2], [1, 2]])
            xy4 = wpool.tile([128, 4], F32)
            nc.vector.tensor_tensor(xy4[:], t3[:], offs_rep, SUB)

            # --- per-point coefficients ------------------------------------
            # bc[:, 0:4] = B' = (2c/RES) xy' ; bc[:, 4:8] = C2 = -c xy'^2
            bc = wpool.tile([128, 8], F32)
            nc.vector.tensor_scalar(
                bc[:, 0:4], xy4[:], 2.0 * INV2S2 / RES, None, MULT
            )
            nc.vector.scalar_tensor_tensor(
                bc[:, 4:8], xy4[:], -INV2S2, xy4[:], MULT, MULT
            )
            nc.vector.tensor_scalar(bc[:, 5:6], bc[:, 5:6], LN_S, None, SUB)
            nc.vector.tensor_scalar(bc[:, 7:8], bc[:, 7:8], LN_S, None, SUB)

            # --- exponent args + exp ---------------------------------------
            gxe = []
            gye = []
            for k in range(2):
                argx = wpool.tile([128, NCOLS], F32, tag=f"argx{k}")
                nc.vector.scalar_tensor_tensor(
                    argx[:], gxi[:], bc[:, 2 * k : 2 * k + 1], cg2x[:], MULT, SUB
                )
                ex = wpool.tile([128, NCOLS], G_DTYPE, tag=f"gxe{k}")
                nc.scalar.activation(
                    ex[:], argx[:], EXP, bias=bc[:, 4 + 2 * k : 5 + 2 * k]
                )
                gxe.append(ex)

                argy = wpool.tile([128, MROWS], F32, tag=f"argy{k}")
                nc.vector.scalar_tensor_tensor(
                    argy[:], gyi[:], bc[:, 2 * k + 1 : 2 * k + 2], cg2y[:],
                    MULT, SUB
                )
                ey = wpool.tile([128, MROWS], G_DTYPE, tag=f"gye{k}")
                nc.scalar.activation(
                    ey[:], argy[:], EXP, bias=bc[:, 5 + 2 * k : 6 + 2 * k]
                )
                gye.append(ey)

            # --- matmul: OUT[m, n] = sum_s Ey[s, m] * Ex[s, n] -------------
            pouts = [
                ppool.tile([128, NCOLS], F32, tag=f"pout{m}", name=f"pout{m}")
                for m in range(2)
            ]
            for k in range(2):
                for m in (1, 0):
                    nc.tensor.matmul(
                        pouts[m][:],
                        gye[k][:, 128 * m : 128 * (m + 1)],
                        gxe[k][:],
                        start=(k == 0),
                        stop=(k == 1),
                        skip_group_check=True,
                    )

            # --- evacuate + store (parallel engines + HWDGE rings) ---------
            out1 = wpool.tile([128, NCOLS], F32, tag="out1")
            nc.vector.tensor_copy(out1[:], pouts[1][:])
            nc.sync.dma_start(out[128:256, :], out1[:])
            out0 = wpool.tile([128, NCOLS], F32, tag="out0")
            nc.scalar.copy(out0[:], pouts[0][:])
            nc.scalar.dma_start(out[0:128, :], out0[:])

    nc.compile()
    return nc


def _static_inputs():
    per_core = []
    for i in range(N_CORES):
        r, c = i // C_BLK, i % C_BLK
        base = np.zeros((1, 16), dtype=np.float32)
        base[0, 12] = (c * NCOLS) / RES
        base[0, 13] = (r * MROWS) / RES
        per_core.append(base)
    return per_core


def _get_cached():
    if "nc" not in _CACHE:
        _CACHE["nc"] = _build_nc()
        _CACHE["static"] = _static_inputs()
    return _CACHE["nc"], _CACHE["static"]


def kernel(control_points: np.ndarray, _trace: bool = False):
    nc, static = _get_cached()
    cp = np.asarray(control_points, dtype=np.float32)
    assert cp.shape == (3, 2)

    in_maps = []
    flat = cp.reshape(-1)
    for i in range(N_CORES):
        row = static[i].copy()
        row[0, 0:6] = flat
        row[0, 6:12] = flat
        in_maps.append(
            {"cpk": np.ascontiguousarray(np.broadcast_to(row, (128, 16)))}
        )

    res = run_bass_kernel_spmd(
        nc, in_maps, core_ids=list(range(N_CORES)), trace=_trace
    )
    _CACHE["last_results"] = res

    full = np.empty((RES, RES), dtype=np.float32)
    for i in range(N_CORES):
        r, c = i // C_BLK, i % C_BLK
        full[r * MROWS : (r + 1) * MROWS, c * NCOLS : (c + 1) * NCOLS] = res.results[
            i
        ]["out"]
    return full


# revision 30
# speedup vs baseline: 1.5644x; 1.0035x over previous
"""Bezier Gaussian-splat raster kernel for 8 Trainium2 NeuronCores.

Reference computation (RES=1024, STEPS=256, SIGMA=0.01):
    curve = bezier(control_points)            # (2, 256)
    Ex[a,s] = exp(-(g[a]-x[s])^2 / (2 sigma^2))   # (1024, 256)
    Ey[b,s] = exp(-(g[b]-y[s])^2 / (2 sigma^2))
    OUT     = (Ey @ Ex^T) / 256               # (1024, 1024)  == raster.T

Sharding: 4 row-blocks x 2 col-blocks = 8 cores. Core i handles output rows
[256*(i//2), +256) and cols [512*(i%2), +512).

Design notes (per core):
  - One tiny input DMA: control points broadcast to 128 partitions plus the
    block offsets, [128, 16] f32. The grid pipeline never touches it: grids
    are iota-generated block-LOCAL indices; the per-core offset is applied to
    the curve instead (x' = x - offs), which is mathematically identical.
  - Bezier basis is computed on device from a [128, 2] iota; curve points are
    elementwise basis*control-point products summed on DVE.
  - Exponent args stay fp32: arg = (2c x'/RES)*j - Square(sqrt(c)/RES * j)
    (+ per-point bias -c x'^2 inside the ACT exp). exp outputs are fp16.
  - The 1/STEPS scale rides the y-side exp biases (-ln S).
  - 256-contraction fp16 matmuls (2 s-chunks x 2 m-chunks, N=512) write the
    final output into PSUM; ACT and DVE evacuate one m-chunk each and the two
    stores go out on the two HWDGE rings in parallel.
"""

import math

import numpy as np

import concourse.bacc as bacc
import concourse.bass as bass
import concourse.mybir as mybir
import concourse.tile as tile
from concourse.bass_utils import run_bass_kernel_spmd

RES = 1024
STEPS = 256
SIGMA = 0.01
INV2S2 = 1.0 / (2.0 * SIGMA * SIGMA)  # 5000.0
SQC = math.sqrt(INV2S2)
LN_S = math.log(STEPS)

R_BLK = 4
C_BLK = 2
MROWS = RES // R_BLK  # 256
NCOLS = RES // C_BLK  # 512
N_CORES = 8

F32 = mybir.dt.float32
F16 = mybir.dt.float16
I16 = mybir.dt.int16

G_DTYPE = F16

_CACHE: dict = {}


def _build_nc() -> bass.Bass:
    # Skip the ~3µs all-engine EVSEM barrier Bass.__init__ emits after its
    # const-AP memsets; our first const-AP use is µs later.
    _orig_barrier = bass.Bass.all_engine_barrier
    bass.Bass.all_engine_barrier = lambda self, **kw: None
    try:
        nc = bacc.Bacc(
            "TRN2",
            target_bir_lowering=False,
            debug=False,
            enable_asserts=False,
            enable_partition_id=False,
        )
    finally:
        bass.Bass.all_engine_barrier = _orig_barrier

    # cols 0:12 block-shifted control points k-duplicated
    # (cp[j,d] - block_offset[d] at col k*6+j*2+d), rest pad.
    cpk = nc.dram_tensor("cpk", [128, 16], F32, kind="ExternalInput").ap()
    out = nc.dram_tensor("out", [MROWS, NCOLS], F32, kind="ExternalOutput").ap()

    MULT = mybir.AluOpType.mult
    ADD = mybir.AluOpType.add
    SUB = mybir.AluOpType.subtract
    EXP = mybir.ActivationFunctionType.Exp
    SQUARE = mybir.ActivationFunctionType.Square

    with tile.TileContext(nc) as tc:
        with (
            tc.tile_pool(name="const", bufs=1) as cpool,
            tc.tile_pool(name="work", bufs=1) as wpool,
            tc.tile_pool(name="ps", bufs=1, space="PSUM") as ppool,
        ):
            # --- the one input DMA, on the ACT HWDGE ring, issued first ----
            cpk_sb = cpool.tile([128, 16], F32)
            nc.scalar.dma_start(cpk_sb[:], cpk)

            # --- early ACT exp-table load trigger --------------------------
            scratch = cpool.tile([128, 2], F32)
            nc.gpsimd.memset(scratch[:], 0.0)
            nc.scalar.activation(scratch[:, 1:2], scratch[:, 0:1], EXP)

            # --- iota grids (int16 indices, block-local) -------------------
            sPk = cpool.tile([128, 2], I16)
            nc.gpsimd.iota(sPk[:], [[128, 2]], base=0, channel_multiplier=1)
            gxi = cpool.tile([128, NCOLS], I16)
            nc.gpsimd.iota(gxi[:], [[1, NCOLS]], base=0, channel_multiplier=0)
            gyi = cpool.tile([128, MROWS], I16)
            nc.gpsimd.iota(gyi[:], [[1, MROWS]], base=0, channel_multiplier=0)

            # --- +c*(j/RES)^2 via ACT Square -------------------------------
            cg2x = wpool.tile([128, NCOLS], F32, tag="cg2x")
            nc.scalar.activation(cg2x[:], gxi[:], SQUARE, scale=SQC / RES)
            cg2y = wpool.tile([128, MROWS], F32, tag="cg2y")
            nc.scalar.activation(cg2y[:], gyi[:], SQUARE, scale=SQC / RES)

            # --- Bezier basis on DVE (s = 128k + p) ------------------------
            # B3[p, 2j+k] = basis_j(s); u = s/255 (linspace), v = s/256
            u = wpool.tile([128, 2], F32)
            nc.vector.tensor_scalar(u[:], sPk[:], 1.0 / 255.0, None, MULT)
            v = wpool.tile([128, 2], F32)
            nc.vector.tensor_scalar(v[:], sPk[:], 1.0 / 256.0, None, MULT)
            su = wpool.tile([128, 2], F32)
            nc.vector.tensor_scalar(su[:], u[:], -1.0, 1.0, MULT, ADD)
            sv = wpool.tile([128, 2], F32)
            nc.vector.tensor_scalar(sv[:], v[:], -1.0, 1.0, MULT, ADD)
            B3 = wpool.tile([128, 6], F32)
            nc.vector.tensor_tensor(B3[:, 0:2], su[:], sv[:], MULT)  # c0
            nc.vector.tensor_tensor(B3[:, 4:6], u[:], v[:], MULT)  # c2
            c02 = wpool.tile([128, 2], F32)
            nc.vector.tensor_tensor(c02[:], B3[:, 0:2], B3[:, 4:6], ADD)
            nc.vector.tensor_scalar(B3[:, 2:4], c02[:], -1.0, 1.0, MULT, ADD)  # c1

            # --- curve points (shifted by block offsets) -------------------
            # prods[p, k*6+j*2+d] = basis_j(s_k) * cp[j, d]
            b3a = B3[:, 0:6]
            in0 = bass.AP(
                b3a.tensor, b3a.offset, [list(b3a.ap[0]), [1, 2], [2, 3], [0, 2]]
            )
            prods = wpool.tile([128, 12], F32)
            nc.vector.tensor_tensor(prods[:], in0, cpk_sb[:, 0:12], MULT)
            # The basis is a partition of unity (c0+c1+c2 = 1), so the host
            # pre-subtracts each core's block offset from the control points;
            # the summed products are directly the block-local curve points.
            pv = prods.rearrange("p (k j d) -> p k j d", k=2, j=3)
            t12 = wpool.tile([128, 4], F32)
            nc.vector.tensor_tensor(t12[:], pv[:, :, 0:1, :], pv[:, :, 1:2, :], ADD)
            # xy4[p, 2k+d] = block-local curve
            xy4 = wpool.tile([128, 4], F32)
            nc.vector.tensor_tensor(xy4[:], t12[:], pv[:, :, 2:3, :], ADD)

            # --- per-point coefficients ------------------------------------
            # bc[:, 0:4] = B' = (2c/RES) xy' ; bc[:, 4:8] = C2 = -c xy'^2
            bc = wpool.tile([128, 8], F32)
            nc.vector.tensor_scalar(
                bc[:, 0:4], xy4[:], 2.0 * INV2S2 / RES, None, MULT
            )
            nc.vector.scalar_tensor_tensor(
                bc[:, 4:8], xy4[:], -INV2S2, xy4[:], MULT, MULT
            )
            nc.vector.tensor_scalar(bc[:, 5:6], bc[:, 5:6], LN_S, None, SUB)
            nc.vector.tensor_scalar(bc[:, 7:8], bc[:, 7:8], LN_S, None, SUB)

            # --- exponent args + exp ---------------------------------------
            gxe = []
            gye = []
            for k in range(2):
                argx = wpool.tile([128, NCOLS], F32, tag=f"argx{k}")
                nc.vector.scalar_tensor_tensor(
                    argx[:], gxi[:], bc[:, 2 * k : 2 * k + 1], cg2x[:], MULT, SUB
                )
                ex = wpool.tile([128, NCOLS], G_DTYPE, tag=f"gxe{k}")
                nc.scalar.activation(
                    ex[:], argx[:], EXP, bias=bc[:, 4 + 2 * k : 5 + 2 * k]
                )
                gxe.append(ex)

                argy = wpool.tile([128, MROWS], F32, tag=f"argy{k}")
                nc.vector.scalar_tensor_tensor(
                    argy[:], gyi[:], bc[:, 2 * k + 1 : 2 * k + 2], cg2y[:],
                    MULT, SUB
                )
                ey = wpool.tile([128, MROWS], G_DTYPE, tag=f"gye{k}")
                nc.scalar.activation(
                    ey[:], argy[:], EXP, bias=bc[:, 5 + 2 * k : 6 + 2 * k]
                )
                gye.append(ey)

            # --- matmul: OUT[m, n] = sum_s Ey[s, m] * Ex[s, n] -------------
            pouts = [
                ppool.tile([128, NCOLS], F32, tag=f"pout{m}", name=f"pout{m}")
                for m in range(2)
            ]
            for k in range(2):
                for m in (1, 0):
                    nc.tensor.matmul(
                        pouts[m][:],
                        gye[k][:, 128 * m : 128 * (m + 1)],
                        gxe[k][:],
                        start=(k == 0),
                        stop=(k == 1),
                        skip_group_check=True,
                    )

            # --- evacuate + store (parallel engines + HWDGE rings) ---------
            out1 = wpool.tile([128, NCOLS], F32, tag="out1")
            nc.vector.tensor_copy(out1[:], pouts[1][:])
            nc.sync.dma_start(out[128:256, :], out1[:])
            out0 = wpool.tile([128, NCOLS], F32, tag="out0")
            nc.scalar.copy(out0[:], pouts[0][:])
            nc.scalar.dma_start(out[0:128, :], out0[:])

    nc.compile()
    return nc


def _get_cached():
    if "nc" not in _CACHE:
        _CACHE["nc"] = _build_nc()
    return _CACHE["nc"]


def kernel(control_points: np.ndarray, _trace: bool = False):
    nc = _get_cached()
    cp = np.asarray(control_points, dtype=np.float32)
    assert cp.shape == (3, 2)

    in_maps = []
    for i in range(N_CORES):
        r, c = i // C_BLK, i % C_BLK
        off = np.array(
            [(c * NCOLS) / RES, (r * MROWS) / RES], dtype=np.float32
        )
        flat = (cp - off[None, :]).reshape(-1).astype(np.float32)
        row = np.zeros((1, 16), dtype=np.float32)
        row[0, 0:6] = flat
        row[0, 6:12] = flat
        in_maps.append(
            {"cpk": np.ascontiguousarray(np.broadcast_to(row, (128, 16)))}
        )

    res = run_bass_kernel_spmd(
        nc, in_maps, core_ids=list(range(N_CORES)), trace=_trace
    )
    _CACHE["last_results"] = res

    full = np.empty((RES, RES), dtype=np.float32)
    for i in range(N_CORES):
        r, c = i // C_BLK, i % C_BLK
        full[r * MROWS : (r + 1) * MROWS, c * NCOLS : (c + 1) * NCOLS] = res.results[
            i
        ]["out"]
    return full


# revision 31
# speedup vs baseline: 1.5655x; 1.0007x over previous
"""Bezier Gaussian-splat raster kernel for 8 Trainium2 NeuronCores.

Reference computation (RES=1024, STEPS=256, SIGMA=0.01):
    curve = bezier(control_points)            # (2, 256)
    Ex[a,s] = exp(-(g[a]-x[s])^2 / (2 sigma^2))   # (1024, 256)
    Ey[b,s] = exp(-(g[b]-y[s])^2 / (2 sigma^2))
    OUT     = (Ey @ Ex^T) / 256               # (1024, 1024)  == raster.T

Sharding: 4 row-blocks x 2 col-blocks = 8 cores. Core i handles output rows
[256*(i//2), +256) and cols [512*(i%2), +512).

Design notes (per core):
  - One tiny input DMA: block-shifted control points broadcast to 128
    partitions, [128, 16] f32. Grids are iota-generated block-LOCAL indices;
    since the Bezier basis is a partition of unity, shifting the control
    points by the block offset shifts the curve identically, so no other
    per-core data is needed.
  - Bezier basis is computed on device from a [128, 2] iota; curve points are
    elementwise basis*control-point products summed on DVE.
  - Exponent args stay fp32: arg = (2c x'/RES)*j - Square(sqrt(c)/RES * j)
    (+ per-point bias -c x'^2 inside the ACT exp). exp outputs are fp16.
  - The 1/STEPS scale rides the y-side exp biases (-ln S).
  - 256-contraction fp16 matmuls (2 s-chunks x 2 m-chunks, N=512) write the
    final output into PSUM; ACT and DVE evacuate one m-chunk each and the two
    stores go out on the two HWDGE rings in parallel.
"""

import math

import numpy as np

import concourse.bacc as bacc
import concourse.bass as bass
import concourse.mybir as mybir
import concourse.tile as tile
from concourse.bass_utils import run_bass_kernel_spmd

RES = 1024
STEPS = 256
SIGMA = 0.01
INV2S2 = 1.0 / (2.0 * SIGMA * SIGMA)  # 5000.0
SQC = math.sqrt(INV2S2)
LN_S = math.log(STEPS)

R_BLK = 4
C_BLK = 2
MROWS = RES // R_BLK  # 256
NCOLS = RES // C_BLK  # 512
N_CORES = 8

F32 = mybir.dt.float32
F16 = mybir.dt.float16
I16 = mybir.dt.int16

G_DTYPE = F16

_CACHE: dict = {}


def _build_nc() -> bass.Bass:
    # Skip the ~3µs all-engine EVSEM barrier Bass.__init__ emits after its
    # const-AP memsets; our first const-AP use is µs later.
    _orig_barrier = bass.Bass.all_engine_barrier
    bass.Bass.all_engine_barrier = lambda self, **kw: None
    try:
        nc = bacc.Bacc(
            "TRN2",
            target_bir_lowering=False,
            debug=False,
            enable_asserts=False,
            enable_partition_id=False,
        )
    finally:
        bass.Bass.all_engine_barrier = _orig_barrier

    # cols 0:12 block-shifted control points k-duplicated
    # (cp[j,d] - block_offset[d] at col k*6+j*2+d), rest pad.
    cpk = nc.dram_tensor("cpk", [128, 16], F32, kind="ExternalInput").ap()
    out = nc.dram_tensor("out", [MROWS, NCOLS], F32, kind="ExternalOutput").ap()

    MULT = mybir.AluOpType.mult
    ADD = mybir.AluOpType.add
    SUB = mybir.AluOpType.subtract
    EXP = mybir.ActivationFunctionType.Exp
    SQUARE = mybir.ActivationFunctionType.Square

    with tile.TileContext(nc) as tc:
        with (
            tc.tile_pool(name="const", bufs=1) as cpool,
            tc.tile_pool(name="work", bufs=1) as wpool,
            tc.tile_pool(name="ps", bufs=1, space="PSUM") as ppool,
        ):
            # --- the one input DMA, on the ACT HWDGE ring, issued first ----
            cpk_sb = cpool.tile([128, 16], F32)
            nc.scalar.dma_start(cpk_sb[:], cpk)

            # --- early ACT exp-table load trigger --------------------------
            scratch = cpool.tile([128, 2], F32)
            nc.gpsimd.memset(scratch[:], 0.0)
            nc.scalar.activation(scratch[:, 1:2], scratch[:, 0:1], EXP)

            # --- iota grids (int16 indices, block-local) -------------------
            sPk = cpool.tile([128, 2], I16)
            nc.gpsimd.iota(sPk[:], [[128, 2]], base=0, channel_multiplier=1)
            gxi = cpool.tile([128, NCOLS], I16)
            nc.gpsimd.iota(gxi[:], [[1, NCOLS]], base=0, channel_multiplier=0)
            gyi = cpool.tile([128, MROWS], I16)
            nc.gpsimd.iota(gyi[:], [[1, MROWS]], base=0, channel_multiplier=0)

            # --- +c*(j/RES)^2 via ACT Square -------------------------------
            cg2x = wpool.tile([128, NCOLS], F32, tag="cg2x")
            nc.scalar.activation(cg2x[:], gxi[:], SQUARE, scale=SQC / RES)
            cg2y = wpool.tile([128, MROWS], F32, tag="cg2y")
            nc.scalar.activation(cg2y[:], gyi[:], SQUARE, scale=SQC / RES)

            # --- Bezier basis on DVE (s = 128k + p) ------------------------
            # B3[p, 2j+k] = basis_j(s); u = s/255 (linspace), v = s/256
            u = wpool.tile([128, 2], F32)
            nc.vector.tensor_scalar(u[:], sPk[:], 1.0 / 255.0, None, MULT)
            v = wpool.tile([128, 2], F32)
            nc.vector.tensor_scalar(v[:], sPk[:], 1.0 / 256.0, None, MULT)
            su = wpool.tile([128, 2], F32)
            nc.vector.tensor_scalar(su[:], u[:], -1.0, 1.0, MULT, ADD)
            sv = wpool.tile([128, 2], F32)
            nc.vector.tensor_scalar(sv[:], v[:], -1.0, 1.0, MULT, ADD)
            B3 = wpool.tile([128, 6], F32)
            nc.vector.tensor_tensor(B3[:, 0:2], su[:], sv[:], MULT)  # c0
            nc.vector.tensor_tensor(B3[:, 4:6], u[:], v[:], MULT)  # c2
            c02 = wpool.tile([128, 2], F32)
            nc.vector.tensor_tensor(c02[:], B3[:, 0:2], B3[:, 4:6], ADD)
            nc.vector.tensor_scalar(B3[:, 2:4], c02[:], -1.0, 1.0, MULT, ADD)  # c1

            # --- curve points (shifted by block offsets) -------------------
            # prods[p, k*6+j*2+d] = basis_j(s_k) * cp[j, d]
            b3a = B3[:, 0:6]
            in0 = bass.AP(
                b3a.tensor, b3a.offset, [list(b3a.ap[0]), [1, 2], [2, 3], [0, 2]]
            )
            prods = wpool.tile([128, 12], F32)
            nc.vector.tensor_tensor(prods[:], in0, cpk_sb[:, 0:12], MULT)
            # The basis is a partition of unity (c0+c1+c2 = 1), so the host
            # pre-subtracts each core's block offset from the control points;
            # the summed products are directly the block-local curve points.
            pv = prods.rearrange("p (k j d) -> p k j d", k=2, j=3)
            t12 = wpool.tile([128, 4], F32)
            nc.vector.tensor_tensor(t12[:], pv[:, :, 0:1, :], pv[:, :, 1:2, :], ADD)
            # xy4[p, 2k+d] = block-local curve
            xy4 = wpool.tile([128, 4], F32)
            nc.vector.tensor_tensor(xy4[:], t12[:], pv[:, :, 2:3, :], ADD)

            # --- per-point coefficients ------------------------------------
            # bc[:, 0:4] = B' = (2c/RES) xy' ; bc[:, 4:8] = C2 = -c xy'^2
            bc = wpool.tile([128, 8], F32)
            nc.vector.tensor_scalar(
                bc[:, 0:4], xy4[:], 2.0 * INV2S2 / RES, None, MULT
            )
            nc.vector.scalar_tensor_tensor(
                bc[:, 4:8], xy4[:], -INV2S2, xy4[:], MULT, MULT
            )
            nc.vector.tensor_scalar(bc[:, 5:6], bc[:, 5:6], LN_S, None, SUB)
            nc.vector.tensor_scalar(bc[:, 7:8], bc[:, 7:8], LN_S, None, SUB)

            # --- exponent args + exp ---------------------------------------
            gxe = []
            gye = []
            for k in range(2):
                argx = wpool.tile([128, NCOLS], F32, tag=f"argx{k}")
                nc.vector.scalar_tensor_tensor(
                    argx[:], gxi[:], bc[:, 2 * k : 2 * k + 1], cg2x[:], MULT, SUB
                )
                ex = wpool.tile([128, NCOLS], G_DTYPE, tag=f"gxe{k}")
                nc.scalar.activation(
                    ex[:], argx[:], EXP, bias=bc[:, 4 + 2 * k : 5 + 2 * k]
                )
                gxe.append(ex)

                argy = wpool.tile([128, MROWS], F32, tag=f"argy{k}")
                nc.vector.scalar_tensor_tensor(
                    argy[:], gyi[:], bc[:, 2 * k + 1 : 2 * k + 2], cg2y[:],
                    MULT, SUB
                )
                ey = wpool.tile([128, MROWS], G_DTYPE, tag=f"gye{k}")
                nc.scalar.activation(
                    ey[:], argy[:], EXP, bias=bc[:, 5 + 2 * k : 6 + 2 * k]
                )
                gye.append(ey)

            # --- matmul: OUT[m, n] = sum_s Ey[s, m] * Ex[s, n] -------------
            pouts = [
                ppool.tile([128, NCOLS], F32, tag=f"pout{m}", name=f"pout{m}")
                for m in range(2)
            ]
            for k in range(2):
                for m in (1, 0):
                    nc.tensor.matmul(
                        pouts[m][:],
                        gye[k][:, 128 * m : 128 * (m + 1)],
                        gxe[k][:],
                        start=(k == 0),
                        stop=(k == 1),
                        skip_group_check=True,
                    )

            # --- evacuate + store (parallel engines + HWDGE rings) ---------
            out1 = wpool.tile([128, NCOLS], F32, tag="out1")
            nc.vector.tensor_copy(out1[:], pouts[1][:])
            nc.sync.dma_start(out[128:256, :], out1[:])
            out0 = wpool.tile([128, NCOLS], F32, tag="out0")
            nc.scalar.copy(out0[:], pouts[0][:])
            nc.scalar.dma_start(out[0:128, :], out0[:])

    nc.compile()
    return nc


def _get_cached():
    if "nc" not in _CACHE:
        _CACHE["nc"] = _build_nc()
    return _CACHE["nc"]


def kernel(control_points: np.ndarray, _trace: bool = False):
    nc = _get_cached()
    cp = np.asarray(control_points, dtype=np.float32)
    assert cp.shape == (3, 2)

    in_maps = []
    for i in range(N_CORES):
        r, c = i // C_BLK, i % C_BLK
        off = np.array(
            [(c * NCOLS) / RES, (r * MROWS) / RES], dtype=np.float32
        )
        flat = (cp - off[None, :]).reshape(-1).astype(np.float32)
        row = np.zeros((1, 16), dtype=np.float32)
        row[0, 0:6] = flat
        row[0, 6:12] = flat
        in_maps.append(
            {"cpk": np.ascontiguousarray(np.broadcast_to(row, (128, 16)))}
        )

    res = run_bass_kernel_spmd(
        nc, in_maps, core_ids=list(range(N_CORES)), trace=_trace
    )
    _CACHE["last_results"] = res

    full = np.empty((RES, RES), dtype=np.float32)
    for i in range(N_CORES):
        r, c = i // C_BLK, i % C_BLK
        full[r * MROWS : (r + 1) * MROWS, c * NCOLS : (c + 1) * NCOLS] = res.results[
            i
        ]["out"]
    return full


# revision 32
# speedup vs baseline: 1.5883x; 1.0146x over previous
"""Bezier Gaussian-splat raster kernel for 8 Trainium2 NeuronCores.

Reference computation (RES=1024, STEPS=256, SIGMA=0.01):
    curve = bezier(control_points)            # (2, 256)
    Ex[a,s] = exp(-(g[a]-x[s])^2 / (2 sigma^2))   # (1024, 256)
    Ey[b,s] = exp(-(g[b]-y[s])^2 / (2 sigma^2))
    OUT     = (Ey @ Ex^T) / 256               # (1024, 1024)  == raster.T

Sharding: 4 row-blocks x 2 col-blocks = 8 cores. Core i handles output rows
[256*(i//2), +256) and cols [512*(i%2), +512).

Design notes (per core):
  - One tiny input DMA: block-shifted control points broadcast to 128
    partitions, [128, 16] f32. Grids are iota-generated block-LOCAL indices;
    since the Bezier basis is a partition of unity, shifting the control
    points by the block offset shifts the curve identically, so no other
    per-core data is needed.
  - Bezier basis is computed on device from a [128, 2] iota; curve points are
    elementwise basis*control-point products summed on DVE.
  - Exponent args stay fp32: arg = (2c x'/RES)*j - Square(sqrt(c)/RES * j)
    (+ per-point bias -c x'^2 inside the ACT exp). exp outputs are fp16.
  - The 1/STEPS scale rides the y-side exp biases (-ln S).
  - 256-contraction fp16 matmuls (2 s-chunks x 2 m-chunks, N=512) write the
    final output into PSUM; ACT and DVE evacuate one m-chunk each and the two
    stores go out on the two HWDGE rings in parallel.
"""

import math

import numpy as np

import concourse.bacc as bacc
import concourse.bass as bass
import concourse.mybir as mybir
import concourse.tile as tile
from concourse.bass_utils import run_bass_kernel_spmd

RES = 1024
STEPS = 256
SIGMA = 0.01
INV2S2 = 1.0 / (2.0 * SIGMA * SIGMA)  # 5000.0
SQC = math.sqrt(INV2S2)
LN_S = math.log(STEPS)

R_BLK = 4
C_BLK = 2
MROWS = RES // R_BLK  # 256
NCOLS = RES // C_BLK  # 512
N_CORES = 8

F32 = mybir.dt.float32
F16 = mybir.dt.float16
I16 = mybir.dt.int16

G_DTYPE = F16

_CACHE: dict = {}


def _build_nc() -> bass.Bass:
    # Skip the ~3µs all-engine EVSEM barrier Bass.__init__ emits after its
    # const-AP memsets; our first const-AP use is µs later.
    _orig_barrier = bass.Bass.all_engine_barrier
    bass.Bass.all_engine_barrier = lambda self, **kw: None
    try:
        nc = bacc.Bacc(
            "TRN2",
            target_bir_lowering=False,
            debug=False,
            enable_asserts=False,
            enable_partition_id=False,
        )
    finally:
        bass.Bass.all_engine_barrier = _orig_barrier

    # cols 0:12 block-shifted control points k-duplicated
    # (cp[j,d] - block_offset[d] at col k*6+j*2+d), rest pad.
    cpk = nc.dram_tensor("cpk", [128, 16], F32, kind="ExternalInput").ap()
    out = nc.dram_tensor("out", [MROWS, NCOLS], F32, kind="ExternalOutput").ap()

    MULT = mybir.AluOpType.mult
    ADD = mybir.AluOpType.add
    SUB = mybir.AluOpType.subtract
    EXP = mybir.ActivationFunctionType.Exp
    SQUARE = mybir.ActivationFunctionType.Square

    with tile.TileContext(nc) as tc:
        with (
            tc.tile_pool(name="const", bufs=1) as cpool,
            tc.tile_pool(name="work", bufs=1) as wpool,
            tc.tile_pool(name="ps", bufs=1, space="PSUM") as ppool,
        ):
            # --- the one input DMA, on the ACT HWDGE ring, issued first ----
            cpk_sb = cpool.tile([128, 16], F32)
            nc.scalar.dma_start(cpk_sb[:], cpk)

            # --- early ACT exp-table load trigger --------------------------
            scratch = cpool.tile([128, 2], F32)
            nc.gpsimd.memset(scratch[:], 0.0)
            nc.scalar.activation(scratch[:, 1:2], scratch[:, 0:1], EXP)

            # --- iota grids (int16 indices, block-local) -------------------
            sPk = cpool.tile([128, 2], I16)
            nc.gpsimd.iota(sPk[:], [[128, 2]], base=0, channel_multiplier=1)
            gxi = cpool.tile([128, NCOLS], I16)
            nc.gpsimd.iota(gxi[:], [[1, NCOLS]], base=0, channel_multiplier=0)
            gyi = cpool.tile([128, MROWS], I16)
            nc.gpsimd.iota(gyi[:], [[1, MROWS]], base=0, channel_multiplier=0)

            # --- +c*(j/RES)^2 via ACT Square -------------------------------
            cg2x = wpool.tile([128, NCOLS], F32, tag="cg2x")
            nc.scalar.activation(cg2x[:], gxi[:], SQUARE, scale=SQC / RES)
            cg2y = wpool.tile([128, MROWS], F32, tag="cg2y")
            nc.scalar.activation(cg2y[:], gyi[:], SQUARE, scale=SQC / RES)

            # --- Bezier basis on DVE (s = 128k + p) ------------------------
            # B3[p, 2j+k] = basis_j(s); u = s/255 (linspace), v = s/256
            u = wpool.tile([128, 2], F32)
            nc.vector.tensor_scalar(u[:], sPk[:], 1.0 / 255.0, None, MULT)
            v = wpool.tile([128, 2], F32)
            nc.vector.tensor_scalar(v[:], sPk[:], 1.0 / 256.0, None, MULT)
            su = wpool.tile([128, 2], F32)
            nc.vector.tensor_scalar(su[:], u[:], -1.0, 1.0, MULT, ADD)
            sv = wpool.tile([128, 2], F32)
            nc.vector.tensor_scalar(sv[:], v[:], -1.0, 1.0, MULT, ADD)
            B3 = wpool.tile([128, 6], F32)
            nc.vector.tensor_tensor(B3[:, 0:2], su[:], sv[:], MULT)  # c0
            nc.vector.tensor_tensor(B3[:, 4:6], u[:], v[:], MULT)  # c2
            c02 = wpool.tile([128, 2], F32)
            nc.vector.tensor_tensor(c02[:], B3[:, 0:2], B3[:, 4:6], ADD)
            nc.vector.tensor_scalar(B3[:, 2:4], c02[:], -1.0, 1.0, MULT, ADD)  # c1

            # --- curve points (shifted by block offsets) -------------------
            # prods[p, k*6+j*2+d] = basis_j(s_k) * cp[j, d]
            b3a = B3[:, 0:6]
            in0 = bass.AP(
                b3a.tensor, b3a.offset, [list(b3a.ap[0]), [1, 2], [2, 3], [0, 2]]
            )
            prods = wpool.tile([128, 12], F32)
            nc.vector.tensor_tensor(prods[:], in0, cpk_sb[:, 0:12], MULT)
            # The basis is a partition of unity (c0+c1+c2 = 1), so the host
            # pre-subtracts each core's block offset from the control points;
            # the summed products are directly the block-local curve points.
            # One reduce over the re-striped (k, d, j) view sums the 3 basis
            # products per coordinate: xy4[p, 2k+d] = block-local curve.
            pa = prods[:, 0:12]
            pv2 = bass.AP(
                pa.tensor, pa.offset, [list(pa.ap[0]), [6, 2], [1, 2], [2, 3]]
            )
            xy4 = wpool.tile([128, 4], F32)
            nc.vector.reduce_sum(xy4[:], pv2, axis=mybir.AxisListType.X)

            # --- per-point coefficients ------------------------------------
            # bc[:, 0:4] = B' = (2c/RES) xy' ; bc[:, 4:8] = C2 = -c xy'^2
            bc = wpool.tile([128, 8], F32)
            nc.vector.tensor_scalar(
                bc[:, 0:4], xy4[:], 2.0 * INV2S2 / RES, None, MULT
            )
            nc.vector.scalar_tensor_tensor(
                bc[:, 4:8], xy4[:], -INV2S2, xy4[:], MULT, MULT
            )
            nc.vector.tensor_scalar(bc[:, 5:6], bc[:, 5:6], LN_S, None, SUB)
            nc.vector.tensor_scalar(bc[:, 7:8], bc[:, 7:8], LN_S, None, SUB)

            # --- exponent args + exp ---------------------------------------
            gxe = []
            gye = []
            for k in range(2):
                argx = wpool.tile([128, NCOLS], F32, tag=f"argx{k}")
                nc.vector.scalar_tensor_tensor(
                    argx[:], gxi[:], bc[:, 2 * k : 2 * k + 1], cg2x[:], MULT, SUB
                )
                ex = wpool.tile([128, NCOLS], G_DTYPE, tag=f"gxe{k}")
                nc.scalar.activation(
                    ex[:], argx[:], EXP, bias=bc[:, 4 + 2 * k : 5 + 2 * k]
                )
                gxe.append(ex)

                argy = wpool.tile([128, MROWS], F32, tag=f"argy{k}")
                nc.vector.scalar_tensor_tensor(
                    argy[:], gyi[:], bc[:, 2 * k + 1 : 2 * k + 2], cg2y[:],
                    MULT, SUB
                )
                ey = wpool.tile([128, MROWS], G_DTYPE, tag=f"gye{k}")
                nc.scalar.activation(
                    ey[:], argy[:], EXP, bias=bc[:, 5 + 2 * k : 6 + 2 * k]
                )
                gye.append(ey)

            # --- matmul: OUT[m, n] = sum_s Ey[s, m] * Ex[s, n] -------------
            pouts = [
                ppool.tile([128, NCOLS], F32, tag=f"pout{m}", name=f"pout{m}")
                for m in range(2)
            ]
            for k in range(2):
                for m in (1, 0):
                    nc.tensor.matmul(
                        pouts[m][:],
                        gye[k][:, 128 * m : 128 * (m + 1)],
                        gxe[k][:],
                        start=(k == 0),
                        stop=(k == 1),
                        skip_group_check=True,
                    )

            # --- evacuate + store (parallel engines + HWDGE rings) ---------
            out1 = wpool.tile([128, NCOLS], F32, tag="out1")
            nc.vector.tensor_copy(out1[:], pouts[1][:])
            nc.sync.dma_start(out[128:256, :], out1[:])
            out0 = wpool.tile([128, NCOLS], F32, tag="out0")
            nc.scalar.copy(out0[:], pouts[0][:])
            nc.scalar.dma_start(out[0:128, :], out0[:])

    nc.compile()
    return nc


def _get_cached():
    if "nc" not in _CACHE:
        _CACHE["nc"] = _build_nc()
    return _CACHE["nc"]


def kernel(control_points: np.ndarray, _trace: bool = False):
    nc = _get_cached()
    cp = np.asarray(control_points, dtype=np.float32)
    assert cp.shape == (3, 2)

    in_maps = []
    for i in range(N_CORES):
        r, c = i // C_BLK, i % C_BLK
        off = np.array(
            [(c * NCOLS) / RES, (r * MROWS) / RES], dtype=np.float32
        )
        flat = (cp - off[None, :]).reshape(-1).astype(np.float32)
        row = np.zeros((1, 16), dtype=np.float32)
        row[0, 0:6] = flat
        row[0, 6:12] = flat
        in_maps.append(
            {"cpk": np.ascontiguousarray(np.broadcast_to(row, (128, 16)))}
        )

    res = run_bass_kernel_spmd(
        nc, in_maps, core_ids=list(range(N_CORES)), trace=_trace
    )
    _CACHE["last_results"] = res

    full = np.empty((RES, RES), dtype=np.float32)
    for i in range(N_CORES):
        r, c = i // C_BLK, i % C_BLK
        full[r * MROWS : (r + 1) * MROWS, c * NCOLS : (c + 1) * NCOLS] = res.results[
            i
        ]["out"]
    return full
